# revision 3
# baseline (speedup 1.0000x reference)
"""Trainium2 Bass kernel for Kaldi LinearResample (16 kHz -> 22.05 kHz) on [8, 960000].

Formulation: out[b, 441*k + p] = sum_i x[b, 320*k - 6 + i] * B[i, p] where
B is the dense [384, 441] polyphase filter-tap matrix (13 taps per phase,
zero elsewhere).  Pure data parallel: one batch row per NeuronCore.

Per core, for each k-tile of 128 blocks:
  DMA in x_ext [128k, 384i] (overlapping windows, contiguous 1536B rows)
  -> 3x PE transpose (128x128, via identity matmul) -> PSUM -> DVE copy -> SBUF
  -> 3x accumulating fp32 matmuls (lhsT = x-transposed chunk, rhs = B chunk)
     into one PSUM bank [128k, 441p]
  -> copy to SBUF -> contiguous DMA out.
"""

import math

import numpy as np

N_IN = 960000
NK = 3000          # output blocks of 441 samples; NK*320 == N_IN
P_PH = 441         # phases per block
IEXT = 384         # padded input window per block: [-6, +378)
IOFF = 6
KT = 128           # k blocks per tile
NKT = (NK + KT - 1) // KT   # 24 tiles; last has 56 blocks
N_OUT = NK * P_PH  # 1323000
N_CORES = 8

_ORIG, _NEW, _LPW = 16000, 22050, 6


def _filter_matrix() -> np.ndarray:
    """Dense [IEXT, 441] tap matrix B with B[fi[p] + IOFF + j, p] = w[p, j]."""
    base = math.gcd(_ORIG, _NEW)
    P = _NEW // base
    cutoff = 0.99 * 0.5 * min(_ORIG, _NEW)
    ww = _LPW / (2.0 * cutoff)
    out_t = np.arange(P, dtype=np.float64) / _NEW
    min_i = np.ceil((out_t - ww) * _ORIG)
    max_i = np.floor((out_t + ww) * _ORIG)
    W = int((max_i - min_i + 1).max())
    j = np.arange(W, dtype=np.float64)
    inp_i = min_i[:, None] + j[None, :]
    dt = inp_i / _ORIG - out_t[:, None]
    w = np.zeros_like(dt)
    inside = np.abs(dt) < ww
    w[inside] = 0.5 * (1.0 + np.cos(2.0 * np.pi * cutoff / _LPW * dt[inside]))
    zero = dt == 0.0
    nz = ~zero
    w[nz] *= np.sin(2.0 * np.pi * cutoff * dt[nz]) / (np.pi * dt[nz])
    w[zero] *= 2.0 * cutoff
    w /= _ORIG
    fi = min_i.astype(np.int64)
    wf = w.astype(np.float32)
    B = np.zeros((IEXT, P), dtype=np.float32)
    for p in range(P):
        for jj in range(W):
            B[fi[p] + IOFF + jj, p] += wf[p, jj]
    return B


_CACHE: dict = {}


def _build():
    if "nc" in _CACHE:
        return _CACHE["nc"]

    import concourse.bass as bass
    import concourse.tile as tile
    from concourse import bacc, masks, mybir

    F32 = mybir.dt.float32

    nc = bacc.Bacc("TRN2", target_bir_lowering=False, debug=False,
                   num_devices=N_CORES)
    x_dram = nc.declare_dram_parameter("waveforms", [N_IN], F32, isOutput=False)
    b_dram = nc.declare_dram_parameter("bfilt", [IEXT, P_PH], F32, isOutput=False)
    o_dram = nc.declare_dram_parameter("out", [N_OUT], F32, isOutput=True)
    xh, bh, oh = x_dram.ap().tensor, b_dram.ap().tensor, o_dram.ap().tensor

    with tile.TileContext(nc) as tc:
        with (
            tc.tile_pool(name="const", bufs=1) as cpool,
            tc.tile_pool(name="xin", bufs=3) as xpool,
            tc.tile_pool(name="xt", bufs=2) as xtpool,
            tc.tile_pool(name="pt", bufs=4, space="PSUM") as ptpool,
            tc.tile_pool(name="pacc", bufs=2, space="PSUM") as paccpool,
            tc.tile_pool(name="osb", bufs=3) as opool,
        ):
            ident = cpool.tile([128, 128], F32)
            masks.make_identity(nc, ident[:])
            bsb = cpool.tile([128, 3 * P_PH], F32)
            for c in range(3):
                nc.sync.dma_start(
                    bsb[:, P_PH * c:P_PH * (c + 1)],
                    b_dram[128 * c:128 * (c + 1), :],
                )

            for kt in range(NKT):
                k0 = kt * KT
                kc = min(KT, NK - k0)
                xe = xpool.tile([128, IEXT], F32)
                if kt == 0:
                    # engine partition access must be 32-aligned; zero a
                    # 32-partition strip, the DMAs below overwrite rows 1..31
                    nc.vector.memset(xe[0:32, 0:IOFF], 0.0)
                    nc.sync.dma_start(
                        xe[:, IOFF:IEXT],
                        bass.AP(xh, 0, [[320, KT], [1, IEXT - IOFF]]),
                    )
                    nc.sync.dma_start(
                        xe[1:KT, 0:IOFF],
                        bass.AP(xh, 320 - IOFF, [[320, KT - 1], [1, IOFF]]),
                    )
                elif kt == NKT - 1:
                    # partition kc-1 (k = NK-1) runs past the input end by 58
                    valid_last = N_IN - (320 * (NK - 1) - IOFF)  # 326
                    # 32-aligned strip covering partition kc-1; the full DMA
                    # below overwrites the valid rows of the strip
                    mb = (kc - 1) // 32 * 32
                    nc.vector.memset(xe[mb:mb + 32, valid_last:IEXT], 0.0)
                    nc.sync.dma_start(
                        xe[0:kc - 1, :],
                        bass.AP(xh, 320 * k0 - IOFF, [[320, kc - 1], [1, IEXT]]),
                    )
                    nc.sync.dma_start(
                        xe[kc - 1:kc, 0:valid_last],
                        bass.AP(xh, 320 * (NK - 1) - IOFF, [[320, 1], [1, valid_last]]),
                    )
                else:
                    nc.sync.dma_start(
                        xe[:, :],
                        bass.AP(xh, 320 * k0 - IOFF, [[320, KT], [1, IEXT]]),
                    )

                xt = xtpool.tile([128, IEXT], F32)
                for c in range(3):
                    pt = ptpool.tile([128, 128], F32)
                    nc.tensor.matmul(
                        pt[:, :kc],
                        xe[:kc, 128 * c:128 * (c + 1)],
                        ident[:kc, :kc],
                        is_transpose=True,
                    )
                    nc.vector.tensor_copy(xt[:, 128 * c:128 * c + kc], pt[:, :kc])

                pacc = paccpool.tile([128, P_PH], F32)
                for c in range(3):
                    nc.tensor.matmul(
                        pacc[:kc, :],
                        xt[:, 128 * c:128 * c + kc],
                        bsb[:, P_PH * c:P_PH * (c + 1)],
                        start=(c == 0),
                        stop=(c == 2),
                    )

                ot = opool.tile([128, P_PH], F32)
                nc.scalar.mul(ot[:kc, :], pacc[:kc, :], 1.0)
                nc.scalar.dma_start(
                    bass.AP(oh, P_PH * k0, [[P_PH, kc], [1, P_PH]]),
                    ot[:kc, :],
                )

    nc.compile()
    _CACHE["nc"] = nc
    return nc


def _run(waveforms: np.ndarray, trace: bool = False):
    from concourse.bass_utils import run_bass_kernel_spmd

    nc = _build()
    bmat = _CACHE.setdefault("bmat", _filter_matrix())
    in_maps = [
        {"waveforms": np.ascontiguousarray(waveforms[b], dtype=np.float32),
         "bfilt": bmat}
        for b in range(N_CORES)
    ]
    res = run_bass_kernel_spmd(nc, in_maps, list(range(N_CORES)), trace=trace)
    out = np.stack([np.asarray(res.results[b]["out"]).reshape(N_OUT)
                    for b in range(N_CORES)])
    return out, res


def kernel(waveforms: np.ndarray) -> np.ndarray:
    out, _ = _run(np.asarray(waveforms))
    return out


# revision 4
# speedup vs baseline: 1.5045x; 1.5045x over previous
"""Trainium2 Bass kernel for Kaldi LinearResample (16 kHz -> 22.05 kHz) on [8, 960000].

Formulation: out[b, 441*k + p] = sum_i x[b, 320*k - 6 + i] * B[i, p] where
B is the dense [384, 441] polyphase filter-tap matrix (13 taps per phase,
zero elsewhere).  Pure data parallel: one batch row per NeuronCore.

Per core, for each k-tile of 128 blocks:
  SWDGE DMA in with f32->bf16 cast: x_ext [128k, 384i] (overlapping windows)
  -> 3x PE transpose (128x128 bf16, via identity matmul) -> PSUM -> DVE -> SBUF
  -> 3x accumulating bf16 matmuls (lhsT = x-transposed chunk, rhs = B chunk,
     each streaming only its structurally-nonzero phase-column range)
     into one f32 PSUM bank [128k, 441p]
  -> copy to SBUF -> contiguous DMA out.
"""

import math

import numpy as np

N_IN = 960000
NK = 3000          # output blocks of 441 samples; NK*320 == N_IN
P_PH = 441         # phases per block
IEXT = 384         # padded input window per block: [-6, +378)
IOFF = 6
KT = 128           # k blocks per tile
NKT = (NK + KT - 1) // KT   # 24 tiles; last has 56 blocks
N_OUT = NK * P_PH  # 1323000
N_CORES = 8

# structurally nonzero phase columns per 128-row chunk of B
COLR = [(0, 176), (160, 352), (337, 441)]

_ORIG, _NEW, _LPW = 16000, 22050, 6


def _filter_matrix() -> np.ndarray:
    """Dense [IEXT, 441] tap matrix B with B[fi[p] + IOFF + j, p] = w[p, j]."""
    base = math.gcd(_ORIG, _NEW)
    P = _NEW // base
    cutoff = 0.99 * 0.5 * min(_ORIG, _NEW)
    ww = _LPW / (2.0 * cutoff)
    out_t = np.arange(P, dtype=np.float64) / _NEW
    min_i = np.ceil((out_t - ww) * _ORIG)
    max_i = np.floor((out_t + ww) * _ORIG)
    W = int((max_i - min_i + 1).max())
    j = np.arange(W, dtype=np.float64)
    inp_i = min_i[:, None] + j[None, :]
    dt = inp_i / _ORIG - out_t[:, None]
    w = np.zeros_like(dt)
    inside = np.abs(dt) < ww
    w[inside] = 0.5 * (1.0 + np.cos(2.0 * np.pi * cutoff / _LPW * dt[inside]))
    zero = dt == 0.0
    nz = ~zero
    w[nz] *= np.sin(2.0 * np.pi * cutoff * dt[nz]) / (np.pi * dt[nz])
    w[zero] *= 2.0 * cutoff
    w /= _ORIG
    fi = min_i.astype(np.int64)
    wf = w.astype(np.float32)
    B = np.zeros((IEXT, P), dtype=np.float32)
    for p in range(P):
        for jj in range(W):
            B[fi[p] + IOFF + jj, p] += wf[p, jj]
    return B


_CACHE: dict = {}


def _build():
    if "nc" in _CACHE:
        return _CACHE["nc"]

    import concourse.bass as bass
    import concourse.tile as tile
    from concourse import bacc, masks, mybir

    F32 = mybir.dt.float32
    BF16 = mybir.dt.bfloat16

    nc = bacc.Bacc("TRN2", target_bir_lowering=False, debug=False,
                   num_devices=N_CORES)
    x_dram = nc.declare_dram_parameter("waveforms", [N_IN], F32, isOutput=False)
    b_dram = nc.declare_dram_parameter("bfilt", [IEXT, P_PH], BF16, isOutput=False)
    o_dram = nc.declare_dram_parameter("out", [N_OUT], F32, isOutput=True)
    xh, oh = x_dram.ap().tensor, o_dram.ap().tensor

    with tile.TileContext(nc) as tc:
        with (
            tc.tile_pool(name="const", bufs=1) as cpool,
            tc.tile_pool(name="xin", bufs=4) as xpool,
            tc.tile_pool(name="xt", bufs=3) as xtpool,
            tc.tile_pool(name="pt", bufs=4, space="PSUM") as ptpool,
            tc.tile_pool(name="pacc", bufs=2, space="PSUM") as paccpool,
            tc.tile_pool(name="osb", bufs=4) as opool,
        ):
            ident = cpool.tile([128, 128], BF16)
            masks.make_identity(nc, ident[:])
            bsb = cpool.tile([128, 3 * P_PH], BF16)
            for c in range(3):
                nc.sync.dma_start(
                    bsb[:, P_PH * c:P_PH * (c + 1)],
                    b_dram[128 * c:128 * (c + 1), :],
                )

            for kt in range(NKT):
                k0 = kt * KT
                kc = min(KT, NK - k0)
                xe = xpool.tile([128, IEXT], BF16)
                if kt == 0:
                    # engine partition access must be 32-aligned; zero a
                    # 32-partition strip, the DMAs below overwrite rows 1..31
                    nc.vector.memset(xe[0:32, 0:IOFF], 0.0)
                    nc.gpsimd.dma_start(
                        xe[:, IOFF:IEXT],
                        bass.AP(xh, 0, [[320, KT], [1, IEXT - IOFF]]),
                    )
                    nc.gpsimd.dma_start(
                        xe[1:KT, 0:IOFF],
                        bass.AP(xh, 320 - IOFF, [[320, KT - 1], [1, IOFF]]),
                    )
                elif kt == NKT - 1:
                    # partition kc-1 (k = NK-1) runs past the input end by 58
                    valid_last = N_IN - (320 * (NK - 1) - IOFF)  # 326
                    # 32-aligned strip covering partition kc-1; the full DMA
                    # below overwrites the valid rows of the strip
                    mb = (kc - 1) // 32 * 32
                    nc.vector.memset(xe[mb:mb + 32, valid_last:IEXT], 0.0)
                    nc.gpsimd.dma_start(
                        xe[0:kc - 1, :],
                        bass.AP(xh, 320 * k0 - IOFF, [[320, kc - 1], [1, IEXT]]),
                    )
                    nc.gpsimd.dma_start(
                        xe[kc - 1:kc, 0:valid_last],
                        bass.AP(xh, 320 * (NK - 1) - IOFF, [[320, 1], [1, valid_last]]),
                    )
                else:
                    nc.gpsimd.dma_start(
                        xe[:, :],
                        bass.AP(xh, 320 * k0 - IOFF, [[320, KT], [1, IEXT]]),
                    )

                xt = xtpool.tile([128, IEXT], BF16)
                for c in range(3):
                    pt = ptpool.tile([128, 128], BF16)
                    nc.tensor.matmul(
                        pt[:, :kc],
                        xe[:kc, 128 * c:128 * (c + 1)],
                        ident[:kc, :kc],
                        is_transpose=True,
                    )
                    nc.vector.tensor_copy(xt[:, 128 * c:128 * c + kc], pt[:, :kc])

                pacc = paccpool.tile([128, P_PH], F32)
                for c in range(3):
                    c0, c1 = COLR[c]
                    nc.tensor.matmul(
                        pacc[:kc, c0:c1],
                        xt[:, 128 * c:128 * c + kc],
                        bsb[:, P_PH * c + c0:P_PH * c + c1],
                        start=(c == 0),
                        stop=(c == 2),
                    )

                ot = opool.tile([128, P_PH], F32)
                nc.scalar.mul(ot[:kc, :], pacc[:kc, :], 1.0)
                nc.scalar.dma_start(
                    bass.AP(oh, P_PH * k0, [[P_PH, kc], [1, P_PH]]),
                    ot[:kc, :],
                )

    nc.compile()
    _CACHE["nc"] = nc
    return nc


def _run(waveforms: np.ndarray, trace: bool = False):
    import ml_dtypes

    from concourse.bass_utils import run_bass_kernel_spmd

    nc = _build()
    if "bmat" not in _CACHE:
        _CACHE["bmat"] = _filter_matrix().astype(ml_dtypes.bfloat16)
    bmat = _CACHE["bmat"]
    in_maps = [
        {"waveforms": np.ascontiguousarray(waveforms[b], dtype=np.float32),
         "bfilt": bmat}
        for b in range(N_CORES)
    ]
    res = run_bass_kernel_spmd(nc, in_maps, list(range(N_CORES)), trace=trace)
    out = np.stack([np.asarray(res.results[b]["out"]).reshape(N_OUT)
                    for b in range(N_CORES)])
    return out, res


def kernel(waveforms: np.ndarray) -> np.ndarray:
    out, _ = _run(np.asarray(waveforms))
    return out


# revision 5
# speedup vs baseline: 1.5589x; 1.0361x over previous
"""Trainium2 Bass kernel for Kaldi LinearResample (16 kHz -> 22.05 kHz) on [8, 960000].

Formulation: out[b, 441*k + p] = sum_i x[b, 320*k - 6 + i] * B[i, p] where
B is the dense [384, 441] polyphase filter-tap matrix (13 taps per phase,
zero elsewhere).  Pure data parallel: one batch row per NeuronCore.

Per core, for each k-tile of 128 blocks:
  HWDGE DMA in: x_ext [128k, 384i] f32 (overlapping windows)
  -> 3x PE transpose (128x128 f32, via identity matmul) -> PSUM
  -> DVE copy-cast f32->bf16 -> SBUF
  -> 3x accumulating bf16 matmuls (lhsT = x-transposed chunk, rhs = B chunk,
     each streaming only its structurally-nonzero phase-column range)
     into one f32 PSUM bank [128k, 441p]
  -> copy to SBUF -> contiguous DMA out.
"""

import math

import numpy as np

N_IN = 960000
NK = 3000          # output blocks of 441 samples; NK*320 == N_IN
P_PH = 441         # phases per block
IEXT = 384         # padded input window per block: [-6, +378)
IOFF = 6
KT = 128           # k blocks per tile
NKT = (NK + KT - 1) // KT   # 24 tiles; last has 56 blocks
N_OUT = NK * P_PH  # 1323000
N_CORES = 8

# structurally nonzero phase columns per 128-row chunk of B
COLR = [(0, 176), (160, 352), (337, 441)]

_ORIG, _NEW, _LPW = 16000, 22050, 6


def _filter_matrix() -> np.ndarray:
    """Dense [IEXT, 441] tap matrix B with B[fi[p] + IOFF + j, p] = w[p, j]."""
    base = math.gcd(_ORIG, _NEW)
    P = _NEW // base
    cutoff = 0.99 * 0.5 * min(_ORIG, _NEW)
    ww = _LPW / (2.0 * cutoff)
    out_t = np.arange(P, dtype=np.float64) / _NEW
    min_i = np.ceil((out_t - ww) * _ORIG)
    max_i = np.floor((out_t + ww) * _ORIG)
    W = int((max_i - min_i + 1).max())
    j = np.arange(W, dtype=np.float64)
    inp_i = min_i[:, None] + j[None, :]
    dt = inp_i / _ORIG - out_t[:, None]
    w = np.zeros_like(dt)
    inside = np.abs(dt) < ww
    w[inside] = 0.5 * (1.0 + np.cos(2.0 * np.pi * cutoff / _LPW * dt[inside]))
    zero = dt == 0.0
    nz = ~zero
    w[nz] *= np.sin(2.0 * np.pi * cutoff * dt[nz]) / (np.pi * dt[nz])
    w[zero] *= 2.0 * cutoff
    w /= _ORIG
    fi = min_i.astype(np.int64)
    wf = w.astype(np.float32)
    B = np.zeros((IEXT, P), dtype=np.float32)
    for p in range(P):
        for jj in range(W):
            B[fi[p] + IOFF + jj, p] += wf[p, jj]
    return B


_CACHE: dict = {}


def _build():
    if "nc" in _CACHE:
        return _CACHE["nc"]

    import concourse.bass as bass
    import concourse.tile as tile
    from concourse import bacc, mybir

    F32 = mybir.dt.float32
    BF16 = mybir.dt.bfloat16

    nc = bacc.Bacc("TRN2", target_bir_lowering=False, debug=False,
                   num_devices=N_CORES)
    x_dram = nc.declare_dram_parameter("waveforms", [N_IN], F32, isOutput=False)
    b_dram = nc.declare_dram_parameter("bfilt", [IEXT, P_PH], BF16, isOutput=False)
    i_dram = nc.declare_dram_parameter("ident", [128, 128], F32, isOutput=False)
    o_dram = nc.declare_dram_parameter("out", [N_OUT], F32, isOutput=True)
    xh, oh = x_dram.ap().tensor, o_dram.ap().tensor

    with tile.TileContext(nc) as tc:
        with (
            tc.tile_pool(name="const", bufs=1) as cpool,
            tc.tile_pool(name="xin", bufs=4) as xpool,
            tc.tile_pool(name="xt", bufs=3) as xtpool,
            tc.tile_pool(name="pt", bufs=4, space="PSUM") as ptpool,
            tc.tile_pool(name="pacc", bufs=2, space="PSUM") as paccpool,
            tc.tile_pool(name="osb", bufs=4) as opool,
        ):
            ident = cpool.tile([128, 128], F32)
            nc.sync.dma_start(ident[:], i_dram[:, :])
            bsb = cpool.tile([128, 3 * P_PH], BF16)
            for c in range(3):
                nc.sync.dma_start(
                    bsb[:, P_PH * c:P_PH * (c + 1)],
                    b_dram[128 * c:128 * (c + 1), :],
                )

            for kt in range(NKT):
                k0 = kt * KT
                kc = min(KT, NK - k0)
                xe = xpool.tile([128, IEXT], F32)
                if kt == 0:
                    # engine partition access must be 32-aligned; zero a
                    # 32-partition strip, the DMAs below overwrite rows 1..31
                    nc.vector.memset(xe[0:32, 0:IOFF], 0.0)
                    nc.sync.dma_start(
                        xe[:, IOFF:IEXT],
                        bass.AP(xh, 0, [[320, KT], [1, IEXT - IOFF]]),
                    )
                    nc.sync.dma_start(
                        xe[1:KT, 0:IOFF],
                        bass.AP(xh, 320 - IOFF, [[320, KT - 1], [1, IOFF]]),
                    )
                elif kt == NKT - 1:
                    # partition kc-1 (k = NK-1) runs past the input end by 58
                    valid_last = N_IN - (320 * (NK - 1) - IOFF)  # 326
                    # 32-aligned strip covering partition kc-1; the full DMA
                    # below overwrites the valid rows of the strip
                    mb = (kc - 1) // 32 * 32
                    nc.vector.memset(xe[mb:mb + 32, valid_last:IEXT], 0.0)
                    nc.sync.dma_start(
                        xe[0:kc - 1, :],
                        bass.AP(xh, 320 * k0 - IOFF, [[320, kc - 1], [1, IEXT]]),
                    )
                    nc.sync.dma_start(
                        xe[kc - 1:kc, 0:valid_last],
                        bass.AP(xh, 320 * (NK - 1) - IOFF, [[320, 1], [1, valid_last]]),
                    )
                else:
                    nc.sync.dma_start(
                        xe[:, :],
                        bass.AP(xh, 320 * k0 - IOFF, [[320, KT], [1, IEXT]]),
                    )

                xt = xtpool.tile([128, IEXT], BF16)
                for c in range(3):
                    pt = ptpool.tile([128, 128], F32)
                    nc.tensor.matmul(
                        pt[:, :kc],
                        xe[:kc, 128 * c:128 * (c + 1)],
                        ident[:kc, :kc],
                        is_transpose=True,
                    )
                    nc.vector.tensor_copy(xt[:, 128 * c:128 * c + kc], pt[:, :kc])

                pacc = paccpool.tile([128, P_PH], F32)
                for c in range(3):
                    c0, c1 = COLR[c]
                    nc.tensor.matmul(
                        pacc[:kc, c0:c1],
                        xt[:, 128 * c:128 * c + kc],
                        bsb[:, P_PH * c + c0:P_PH * c + c1],
                        start=(c == 0),
                        stop=(c == 2),
                    )

                ot = opool.tile([128, P_PH], F32)
                nc.scalar.mul(ot[:kc, :], pacc[:kc, :], 1.0)
                nc.scalar.dma_start(
                    bass.AP(oh, P_PH * k0, [[P_PH, kc], [1, P_PH]]),
                    ot[:kc, :],
                )

    nc.compile()
    _CACHE["nc"] = nc
    return nc


def _run(waveforms: np.ndarray, trace: bool = False):
    import ml_dtypes

    from concourse.bass_utils import run_bass_kernel_spmd

    nc = _build()
    if "bmat" not in _CACHE:
        _CACHE["bmat"] = _filter_matrix().astype(ml_dtypes.bfloat16)
        _CACHE["ident"] = np.eye(128, dtype=np.float32)
    bmat, idm = _CACHE["bmat"], _CACHE["ident"]
    in_maps = [
        {"waveforms": np.ascontiguousarray(waveforms[b], dtype=np.float32),
         "bfilt": bmat, "ident": idm}
        for b in range(N_CORES)
    ]
    res = run_bass_kernel_spmd(nc, in_maps, list(range(N_CORES)), trace=trace)
    out = np.stack([np.asarray(res.results[b]["out"]).reshape(N_OUT)
                    for b in range(N_CORES)])
    return out, res


def kernel(waveforms: np.ndarray) -> np.ndarray:
    out, _ = _run(np.asarray(waveforms))
    return out


# revision 6
# speedup vs baseline: 1.7105x; 1.0973x over previous
"""Trainium2 Bass kernel for Kaldi LinearResample (16 kHz -> 22.05 kHz) on [8, 960000].

Formulation: out[b, 441*k + p] = sum_i x[b, 320*k - 6 + i] * B[i, p] where
B is the dense [384, 441] polyphase filter-tap matrix (13 taps per phase,
zero elsewhere).  Pure data parallel: one batch row per NeuronCore.

Per core, for each super-tile of 256 blocks (2 x 128):
  one HWDGE 3D DMA in: x_ext [128k, 2, 384i] f32 (overlapping windows)
  -> 6x PE transpose (128x128 f32, via identity matmul) -> PSUM
  -> DVE copy-cast f32->bf16 -> SBUF
  -> 2 sets of 3 accumulating bf16 matmuls (lhsT = x-transposed chunk,
     rhs = B chunk, each streaming only its nonzero phase-column range)
     into f32 PSUM banks [128k, 441p]
  -> ACT copy to SBUF -> one contiguous 3D DMA out (900KB).
"""

import math

import numpy as np

N_IN = 960000
NK = 3000          # output blocks of 441 samples; NK*320 == N_IN
P_PH = 441         # phases per block
IEXT = 384         # padded input window per block: [-6, +378)
IOFF = 6
KT = 128           # k blocks per tile
NKT = (NK + KT - 1) // KT   # 24 tiles; last has 56 blocks
N_OUT = NK * P_PH  # 1323000
N_CORES = 8

# structurally nonzero phase columns per 128-row chunk of B
COLR = [(0, 176), (160, 352), (337, 441)]

_ORIG, _NEW, _LPW = 16000, 22050, 6


def _filter_matrix() -> np.ndarray:
    """Dense [IEXT, 441] tap matrix B with B[fi[p] + IOFF + j, p] = w[p, j]."""
    base = math.gcd(_ORIG, _NEW)
    P = _NEW // base
    cutoff = 0.99 * 0.5 * min(_ORIG, _NEW)
    ww = _LPW / (2.0 * cutoff)
    out_t = np.arange(P, dtype=np.float64) / _NEW
    min_i = np.ceil((out_t - ww) * _ORIG)
    max_i = np.floor((out_t + ww) * _ORIG)
    W = int((max_i - min_i + 1).max())
    j = np.arange(W, dtype=np.float64)
    inp_i = min_i[:, None] + j[None, :]
    dt = inp_i / _ORIG - out_t[:, None]
    w = np.zeros_like(dt)
    inside = np.abs(dt) < ww
    w[inside] = 0.5 * (1.0 + np.cos(2.0 * np.pi * cutoff / _LPW * dt[inside]))
    zero = dt == 0.0
    nz = ~zero
    w[nz] *= np.sin(2.0 * np.pi * cutoff * dt[nz]) / (np.pi * dt[nz])
    w[zero] *= 2.0 * cutoff
    w /= _ORIG
    fi = min_i.astype(np.int64)
    wf = w.astype(np.float32)
    B = np.zeros((IEXT, P), dtype=np.float32)
    for p in range(P):
        for jj in range(W):
            B[fi[p] + IOFF + jj, p] += wf[p, jj]
    return B


_CACHE: dict = {}


def _build():
    if "nc" in _CACHE:
        return _CACHE["nc"]

    import concourse.bass as bass
    import concourse.tile as tile
    from concourse import bacc, mybir

    F32 = mybir.dt.float32
    BF16 = mybir.dt.bfloat16

    nc = bacc.Bacc("TRN2", target_bir_lowering=False, debug=False,
                   num_devices=N_CORES)
    x_dram = nc.declare_dram_parameter("waveforms", [N_IN], F32, isOutput=False)
    b_dram = nc.declare_dram_parameter("bfilt", [IEXT, P_PH], BF16, isOutput=False)
    i_dram = nc.declare_dram_parameter("ident", [128, 128], F32, isOutput=False)
    o_dram = nc.declare_dram_parameter("out", [N_OUT], F32, isOutput=True)
    xh = x_dram.ap().tensor
    bh = b_dram.ap().tensor
    oh = o_dram.ap().tensor

    NPAIR = NKT // 2  # 12 super-tiles of (up to) 256 blocks

    with tile.TileContext(nc) as tc:
        with (
            tc.tile_pool(name="const", bufs=1) as cpool,
            tc.tile_pool(name="xin", bufs=3) as xpool,
            tc.tile_pool(name="xt", bufs=3) as xtpool,
            tc.tile_pool(name="pt", bufs=4, space="PSUM") as ptpool,
            tc.tile_pool(name="pacc", bufs=3, space="PSUM") as paccpool,
            tc.tile_pool(name="osb", bufs=3) as opool,
        ):
            # constants on the scalar HWDGE ring so the first x DMA
            # (sync ring) is not queued behind them
            ident = cpool.tile([128, 128], F32)
            nc.scalar.dma_start(ident[:], i_dram[:, :])
            bsb = cpool.tile([128, 3 * P_PH], BF16)
            nc.scalar.dma_start(
                bsb[:],
                bass.AP(bh, 0, [[P_PH, 128], [P_PH * 128, 3], [1, P_PH]]),
            )

            for pr in range(NPAIR):
                k0 = pr * 2 * KT
                xe = xpool.tile([128, 2 * IEXT], F32)
                if pr == 0:
                    # block 0's window starts at x[-6]; zero-fill the strip,
                    # DMAs below overwrite rows 1..31 of it
                    nc.vector.memset(xe[0:32, 0:IOFF], 0.0)
                    nc.sync.dma_start(
                        xe[:, IOFF:IEXT],
                        bass.AP(xh, 0, [[320, KT], [1, IEXT - IOFF]]),
                    )
                    nc.sync.dma_start(
                        xe[1:KT, 0:IOFF],
                        bass.AP(xh, 320 - IOFF, [[320, KT - 1], [1, IOFF]]),
                    )
                    nc.sync.dma_start(
                        xe[:, IEXT:2 * IEXT],
                        bass.AP(xh, 320 * KT - IOFF, [[320, KT], [1, IEXT]]),
                    )
                elif pr == NPAIR - 1:
                    # second window: only 56 blocks; block 2999 (row 55) runs
                    # 58 samples past the input end
                    valid_last = N_IN - (320 * (NK - 1) - IOFF)  # 326
                    mb = (NK - k0 - KT - 1) // 32 * 32  # 32
                    nc.vector.memset(
                        xe[mb:mb + 32, IEXT + valid_last:2 * IEXT], 0.0)
                    nc.sync.dma_start(
                        xe[:, 0:IEXT],
                        bass.AP(xh, 320 * k0 - IOFF, [[320, KT], [1, IEXT]]),
                    )
                    nc.sync.dma_start(
                        xe[0:55, IEXT:2 * IEXT],
                        bass.AP(xh, 320 * (k0 + KT) - IOFF, [[320, 55], [1, IEXT]]),
                    )
                    nc.sync.dma_start(
                        xe[55:56, IEXT:IEXT + valid_last],
                        bass.AP(xh, 320 * (NK - 1) - IOFF, [[320, 1], [1, valid_last]]),
                    )
                else:
                    nc.sync.dma_start(
                        xe[:],
                        bass.AP(xh, 320 * k0 - IOFF,
                                [[320, KT], [320 * KT, 2], [1, IEXT]]),
                    )

                ot = opool.tile([128, 2 * P_PH], F32)
                for half in range(2):
                    kh0 = k0 + half * KT
                    kc = min(KT, NK - kh0)
                    xt = xtpool.tile([128, IEXT], BF16)
                    for c in range(3):
                        pt = ptpool.tile([128, 128], F32)
                        nc.tensor.matmul(
                            pt[:, :kc],
                            xe[:kc, IEXT * half + 128 * c:IEXT * half + 128 * (c + 1)],
                            ident[:kc, :kc],
                            is_transpose=True,
                        )
                        nc.vector.tensor_copy(xt[:, 128 * c:128 * c + kc],
                                              pt[:, :kc])

                    pacc = paccpool.tile([128, P_PH], F32)
                    for c in range(3):
                        c0, c1 = COLR[c]
                        nc.tensor.matmul(
                            pacc[:kc, c0:c1],
                            xt[:, 128 * c:128 * c + kc],
                            bsb[:, P_PH * c + c0:P_PH * c + c1],
                            start=(c == 0),
                            stop=(c == 2),
                        )

                    nc.scalar.mul(ot[:kc, P_PH * half:P_PH * (half + 1)],
                                  pacc[:kc, :], 1.0)

                if pr < NPAIR - 1:
                    nc.scalar.dma_start(
                        bass.AP(oh, P_PH * k0,
                                [[P_PH, KT], [P_PH * KT, 2], [1, P_PH]]),
                        ot[:],
                    )
                else:
                    nc.scalar.dma_start(
                        bass.AP(oh, P_PH * k0, [[P_PH, KT], [1, P_PH]]),
                        ot[:, 0:P_PH],
                    )
                    nc.scalar.dma_start(
                        bass.AP(oh, P_PH * (k0 + KT), [[P_PH, 56], [1, P_PH]]),
                        ot[0:56, P_PH:2 * P_PH],
                    )

    nc.compile()
    _CACHE["nc"] = nc
    return nc


def _run(waveforms: np.ndarray, trace: bool = False):
    import ml_dtypes

    from concourse.bass_utils import run_bass_kernel_spmd

    nc = _build()
    if "bmat" not in _CACHE:
        _CACHE["bmat"] = _filter_matrix().astype(ml_dtypes.bfloat16)
        _CACHE["ident"] = np.eye(128, dtype=np.float32)
    bmat, idm = _CACHE["bmat"], _CACHE["ident"]
    in_maps = [
        {"waveforms": np.ascontiguousarray(waveforms[b], dtype=np.float32),
         "bfilt": bmat, "ident": idm}
        for b in range(N_CORES)
    ]
    res = run_bass_kernel_spmd(nc, in_maps, list(range(N_CORES)), trace=trace)
    out = np.stack([np.asarray(res.results[b]["out"]).reshape(N_OUT)
                    for b in range(N_CORES)])
    return out, res


def kernel(waveforms: np.ndarray) -> np.ndarray:
    out, _ = _run(np.asarray(waveforms))
    return out
